# revision 1
# baseline (speedup 1.0000x reference)
"""Self-contained Trainium2 Bass kernel for nn_GNN_75436805587134.

kernel(**inputs) -> np.ndarray [1024, 1]

Strategy: dst-sharded message passing across 8 NeuronCores; bf16-replicated
node-state table with per-layer AllGather; 4-bank dma_gather for h[src] rows;
edge aggregation as one-hot S-tile matmuls on the PE (norm weights folded);
LayerNorm gammas folded into FiLM tables; residual kept fp32 per-core;
input featurization + graph pooling + head on host.
"""
import sys
for _p in ("/opt/trn_rl_repo",):
    if _p not in sys.path:
        sys.path.insert(0, _p)
import numpy as np
import ml_dtypes

import concourse.bass as bass
import concourse.bacc as bacc
import concourse.tile as tile
import concourse.mybir as mybir
import concourse.bass_utils as bass_utils

bf16 = ml_dtypes.bfloat16



N, E, B = 100000, 400000, 1024
NF, EF, H, C, D = 32, 16, 256, 256, 6
LN_EPS = 1e-5
NC = 8
RPC = N // NC                # 12500 real rows per core
NT = 98                      # dst tiles per core (98*128 = 12544)
TR = NT * 128                # 12544 padded rows per core
TROWS = NC * TR              # 100352 table rows
BANKS = 4
BANK = TROWS // BANKS        # 25088
WLEN = 4                     # dst-tiles per window
NW = (NT + WLEN - 1) // WLEN  # 25 windows
MAX_TILES_PER_CALL = 8


def trow_of(v):
    """global node id -> padded table row"""
    k = v // RPC
    return k * TR + (v - k * RPC)


def build_edge_structure(src, dst):
    """Returns uniform SPMD structure + per-core tile data."""
    src = np.asarray(src).astype(np.int64)
    dst = np.asarray(dst).astype(np.int64)
    deg_out = np.maximum(np.bincount(src, minlength=N), 1.0)
    deg_in = np.maximum(np.bincount(dst, minlength=N), 1.0)
    no = deg_out ** -0.5
    ni = deg_in ** -0.5
    w_edge = (no[src] * ni[dst]).astype(np.float32)
    ratio = (1.0 / deg_in[dst] / w_edge).astype(np.float32)  # w2/w per edge

    trow = trow_of(src)
    bank = trow // BANK
    brow = trow % BANK

    core = dst // RPC
    dloc = dst - core * RPC
    t_of_e = dloc // 128
    drel = dloc - t_of_e * 128

    # counts per (core, t, bank)
    cnt = np.zeros((NC, NT, BANKS), np.int64)
    np.add.at(cnt, (core, t_of_e, bank), 1)
    ntiles_tb = np.maximum(np.ceil(cnt / 128).astype(np.int64).max(axis=0), 1)  # [NT, BANKS]

    # tile order: for w: for b: for t in w: range(ntiles_tb[t, b])
    tile_t = []      # target dst-tile per tile
    tile_b = []
    tile_first = []  # True if first tile of its dst-tile within its window
    call_list = []   # (bank, tile_start, n_tiles) uniform
    for w in range(NW):
        ts = range(w * WLEN, min((w + 1) * WLEN, NT))
        first_seen = set()
        for b in range(BANKS):
            run_start = len(tile_t)
            for t in ts:
                for i in range(ntiles_tb[t, b]):
                    tile_first.append(t not in first_seen)
                    first_seen.add(t)
                    tile_t.append(t)
                    tile_b.append(b)
            n = len(tile_t) - run_start
            s = run_start
            while n > 0:
                c = min(n, MAX_TILES_PER_CALL)
                call_list.append((b, s, c))
                s += c
                n -= c
    ntot = len(tile_t)
    tile_t = np.array(tile_t)
    tile_b = np.array(tile_b)
    tile_first = np.array(tile_first)

    # per-core per-tile data
    # order edges of core k by (t, bank, dloc)
    gidx = np.zeros((NC, ntot, 128), np.int16)
    sval = np.zeros((NC, ntot, 128), np.float32)   # folded w (0 for pads)
    srel = np.zeros((NC, ntot, 128), np.int16)     # dst col in S tile
    eord = np.full((NC, ntot, 128), -1, np.int64)  # original edge id (-1 pad)

    # tile slot index for (t, b): starting tile index
    tile_start_of = {}
    for i, (t, b) in enumerate(zip(tile_t, tile_b)):
        tile_start_of.setdefault((t, b), i)

    order = np.lexsort((drel, bank, t_of_e, core))
    e_sorted = order
    key_core = core[order]
    key_t = t_of_e[order]
    key_b = bank[order]
    # group boundaries
    import itertools
    idx = 0
    for (k, t, b), grp in itertools.groupby(
            range(len(order)), key=lambda i: (key_core[i], key_t[i], key_b[i])):
        grp = list(grp)
        eids = e_sorted[grp]
        base = tile_start_of[(t, b)]
        for j, e in enumerate(eids):
            ti = base + j // 128
            jj = j % 128
            gidx[k, ti, jj] = brow[e]
            sval[k, ti, jj] = w_edge[e]
            srel[k, ti, jj] = drel[e]
            eord[k, ti, jj] = e
    return dict(ntiles_tb=ntiles_tb, tile_t=tile_t, tile_b=tile_b,
                tile_first=tile_first, call_list=call_list, ntot=ntot,
                gidx=gidx, sval=sval, srel=srel, eord=eord,
                no=no, ni=ni, deg_in=deg_in, ratio=ratio)


def wrap_idx(idx_tiles):
    """[ntot,128] int16 -> [128, ntot*8] wrapped+replicated layout"""
    ntot = idx_tiles.shape[0]
    out = np.zeros((128, ntot * 8), np.int16)
    for ti in range(ntot):
        w = idx_tiles[ti].reshape(8, 16).T  # [16, 8]
        out[:, ti * 8:(ti + 1) * 8] = np.tile(w, (8, 1))
    return out


def build_S(st):
    """[NC, ntot, 128, 128] bf16 one-hot*weight"""
    NCn, ntot = st["sval"].shape[0], st["ntot"]
    S = np.zeros((NCn, ntot, 128, 128), bf16)
    j = np.arange(128)
    for k in range(NCn):
        for ti in range(ntot):
            S[k, ti, j, st["srel"][k, ti]] = st["sval"][k, ti].astype(bf16)
            # pad rows have sval 0 -> harmless entry at col srel=0
    return S


def prep_weights(inp):
    """Fold LN gammas into film tables; build device weight arrays."""
    g1 = np.asarray(inp["ln2_g"], np.float32)    # layer path uses ln2/film2/conv2
    b1 = np.asarray(inp["ln2_b"], np.float32)
    fw = np.asarray(inp["film2_w"], np.float32)  # [D, C, 2H]
    fb = np.asarray(inp["film2_b"], np.float32)  # [D, 2H]
    film_tab = np.zeros((D, C + 1, 2 * H), np.float32)
    for i in range(D):
        gam_w = fw[i, :, :H]; bet_w = fw[i, :, H:]
        gam_b = fb[i, :H]; bet_b = fb[i, H:]
        gl = g1[i]; bl = b1[i]
        # Gamma_eff = gl*(1+gamma);  Beta_eff = bl*(1+gamma) + beta
        film_tab[i, :C, :H] = gam_w * gl[None, :]
        film_tab[i, C, :H] = gl * (1.0 + gam_b)
        film_tab[i, :C, H:] = bet_w + gam_w * bl[None, :]
        film_tab[i, C, H:] = bet_b + bl * (1.0 + gam_b)
    return film_tab


def sincos_emb(t):
    half = 64
    freqs = np.exp(-np.log(1000.0) * np.arange(half, dtype=np.float32) / half)
    a = (np.asarray(t, np.float32) * 1000.0)[:, None] * freqs[None, :]
    return np.concatenate([np.sin(a), np.cos(a)], axis=-1).astype(np.float32)  # [B,128]


def host_pool_head(h6_full_real, n_index, head_w, head_b):
    """h6_full_real: [N, H] f32 (real rows). Returns [B, 1]."""
    n_index = np.asarray(n_index).astype(np.int64)
    cnt = np.maximum(np.bincount(n_index, minlength=B), 1.0)
    pooled = np.zeros((B, H), np.float64)
    np.add.at(pooled, n_index, h6_full_real.astype(np.float64))
    g_mean = (pooled / cnt[:, None]).astype(np.float32)
    return g_mean @ np.asarray(head_w, np.float32) + np.asarray(head_b, np.float32)


def host_input_aggregates(inp, st):
    """Exact f32 input-stage aggregation per core -> [NC][64, TR] bf16 (transposed)."""
    src = np.asarray(inp["src"]).astype(np.int64)
    dst = np.asarray(inp["dst"]).astype(np.int64)
    w_e = (st["no"][src] * st["ni"][dst]).astype(np.float32)
    nx = np.asarray(inp["node_x"], np.float32)
    agg_x = np.zeros((N, NF), np.float32)
    np.add.at(agg_x, dst, w_e[:, None] * nx[src])
    ratio = st["ratio"]
    ee = np.asarray(inp["edge_e"], np.float32) * ratio[:, None]
    e_aug = np.concatenate([ee, ratio[:, None]], 1)
    agg_e = np.zeros((N, 17), np.float32)
    np.add.at(agg_e, dst, w_e[:, None] * e_aug)
    outs = []
    for k in range(NC):
        a = np.zeros((64, TR), np.float32)
        a[0:NF, :RPC] = agg_x[k * RPC:(k + 1) * RPC].T
        a[32:49, :RPC] = agg_e[k * RPC:(k + 1) * RPC].T
        outs.append(a.astype(bf16))
    return outs


class _G:
    pass


G = _G()
for _n in ("N", "E", "B", "NF", "EF", "H", "C", "D", "LN_EPS", "NC", "RPC", "NT",
           "TR", "TROWS", "BANKS", "BANK", "WLEN", "NW", "MAX_TILES_PER_CALL",
           "trow_of", "build_edge_structure", "wrap_idx", "build_S",
           "prep_weights", "sincos_emb", "host_pool_head", "host_input_aggregates"):
    setattr(G, _n, globals()[_n])


def kernel(**inputs):
    out, _res, _h6 = run(inputs, trace=False)
    return out.astype(np.float32)

dt = mybir.dt
AF = mybir.ActivationFunctionType
ALU = mybir.AluOpType
NCORES = G.NC


def build(st, nlayers=6, dbg=False):
    ntot = st["ntot"]
    tile_t = st["tile_t"]
    tile_first = st["tile_first"]
    call_list = st["call_list"]
    last_tile_of = {}
    for i, t in enumerate(tile_t):
        last_tile_of[int(t)] = i

    nc = bacc.Bacc("TRN2", target_bir_lowering=False, debug=False,
                   enable_asserts=False, num_devices=NCORES, num_swdge_queues=4)

    # ---------- I/O ----------
    xtab = nc.dram_tensor("xtab", [G.TROWS, 128], dt.bfloat16, kind="ExternalInput").ap()
    S_in = nc.dram_tensor("S_in", [128, ntot * 128], dt.bfloat16, kind="ExternalInput").ap()
    eaug = nc.dram_tensor("eaug", [128, ntot * 32], dt.bfloat16, kind="ExternalInput").ap()
    gidx = nc.dram_tensor("gidx", [128, ntot * 8], dt.int16, kind="ExternalInput").ap()
    nidx = nc.dram_tensor("nidx", [128, G.NT * 8], dt.int16, kind="ExternalInput").ap()
    embT = nc.dram_tensor("embT", [128, G.B], dt.bfloat16, kind="ExternalInput").ap()
    w1 = nc.dram_tensor("w1", [128, 256], dt.bfloat16, kind="ExternalInput").ap()
    w2 = nc.dram_tensor("w2", [256, 256], dt.bfloat16, kind="ExternalInput").ap()
    tb1 = nc.dram_tensor("tb1", [128, 2], dt.float32, kind="ExternalInput").ap()
    tb2 = nc.dram_tensor("tb2", [128, 2], dt.float32, kind="ExternalInput").ap()
    film = nc.dram_tensor("film", [6 * 257, 512], dt.bfloat16, kind="ExternalInput").ap()
    wconv = nc.dram_tensor("wconv", [6 * 256, 256], dt.bfloat16, kind="ExternalInput").ap()
    bconv = nc.dram_tensor("bconv", [6, 256], dt.bfloat16, kind="ExternalInput").ap()
    win_w = nc.dram_tensor("win_w", [32, 256], dt.bfloat16, kind="ExternalInput").ap()
    we_w = nc.dram_tensor("we_w", [32, 256], dt.bfloat16, kind="ExternalInput").ap()
    inb = nc.dram_tensor("inb", [1, 256], dt.bfloat16, kind="ExternalInput").ap()
    axe = nc.dram_tensor("axe", [64, G.TR], dt.bfloat16, kind="ExternalInput").ap()
    h6_out = nc.dram_tensor("h6_out", [G.TR, 256], dt.float32, kind="ExternalOutput").ap()
    gbd_out = nc.dram_tensor("gbd_out", [G.B, 512], dt.bfloat16, kind="ExternalOutput").ap() if dbg else None

    # ---------- internal DRAM ----------
    hA = nc.dram_tensor("hA", [G.TR, 256], dt.float32, kind="Internal").ap()
    hB = nc.dram_tensor("hB", [G.TR, 256], dt.float32, kind="Internal").ap()
    agin = [nc.dram_tensor(f"agin{i}", [G.TR, 256], dt.bfloat16, kind="Internal").ap()
            for i in range(2)]
    tabs = [nc.dram_tensor(f"tab{i}", [G.TROWS, 256], dt.bfloat16, kind="Internal",
                           addr_space="Shared").ap() for i in range(2)]
    gbd = nc.dram_tensor("gbd", [6 * G.B, 512], dt.bfloat16, kind="Internal").ap()

    RG = [list(range(NCORES))]

    def windows_of():
        out = []
        ci = 0
        for w in range(G.NW):
            ts = list(range(w * G.WLEN, min((w + 1) * G.WLEN, G.NT)))
            calls = []
            while ci < len(call_list):
                b, a, n = call_list[ci]
                if tile_t[a] // G.WLEN != w:
                    break
                calls.append((b, a, n))
                ci += 1
            out.append((w, ts, calls))
        return out

    WINS = windows_of()

    with tile.TileContext(nc) as tc:
        with tc.tile_pool(name="const", bufs=1) as constp, \
             tc.tile_pool(name="wpool", bufs=2) as wpool, \
             tc.tile_pool(name="gpool", bufs=10) as gpool, \
             tc.tile_pool(name="spool", bufs=3) as spool, \
             tc.tile_pool(name="gbpool", bufs=4) as gbpool, \
             tc.tile_pool(name="work", bufs=4) as work, \
             tc.tile_pool(name="tiny", bufs=4) as tiny, \
             tc.tile_pool(name="psZ", bufs=2, space="PSUM") as psZ:

            # ---- resident constants ----
            gidx_sb = constp.tile([128, ntot * 8], dt.int16)
            nc.sync.dma_start(gidx_sb[:], gidx[:])
            nidx_sb = constp.tile([128, G.NT * 8], dt.int16)
            nc.sync.dma_start(nidx_sb[:], nidx[:])
            ones1 = constp.tile([1, 128], dt.bfloat16)
            nc.vector.memset(ones1[:], 1.0)
            win_sb = constp.tile([32, 256], dt.bfloat16)
            nc.sync.dma_start(win_sb[:], win_w[:])
            we_sb = constp.tile([32, 256], dt.bfloat16)
            nc.sync.dma_start(we_sb[:], we_w[:])
            inb_sb = constp.tile([1, 256], dt.bfloat16)
            nc.sync.dma_start(inb_sb[:], inb[:])
            epsc = constp.tile([128, 1], dt.float32)
            nc.vector.memset(epsc[:], G.LN_EPS)

            # ---- phase A: cond -> gamma/beta tables ----
            with tc.tile_pool(name="psCond", bufs=2, space="PSUM") as psC:
                embT_sb = constp.tile([128, G.B], dt.bfloat16)
                nc.sync.dma_start(embT_sb[:], embT[:])
                w1_sb = constp.tile([128, 256], dt.bfloat16)
                nc.sync.dma_start(w1_sb[:], w1[:])
                w2_sb = constp.tile([128, 2, 256], dt.bfloat16)
                nc.sync.dma_start(w2_sb[:], w2[:].rearrange("(a p) n -> p a n", p=128))
                tb1_sb = constp.tile([128, 2], dt.float32)
                nc.sync.dma_start(tb1_sb[:], tb1[:])
                tb2_sb = constp.tile([128, 2], dt.float32)
                nc.sync.dma_start(tb2_sb[:], tb2[:])

                c1T = constp.tile([128, 2, G.B], dt.bfloat16)  # [part, oc, t]
                for oc in range(2):
                    for tb in range(2):
                        ps = psC.tile([128, 512], dt.float32, tag="condps", space="PSUM")
                        nc.tensor.matmul(ps[:], w1_sb[:, oc * 128:(oc + 1) * 128],
                                         embT_sb[:, tb * 512:(tb + 1) * 512],
                                         start=True, stop=True)
                        nc.scalar.activation(c1T[:, oc, tb * 512:(tb + 1) * 512], ps[:],
                                             AF.Silu, bias=tb1_sb[:, oc:oc + 1])
                c2T = constp.tile([128, 2, G.B], dt.bfloat16)
                for oc in range(2):
                    for tb in range(2):
                        ps = psC.tile([128, 512], dt.float32, tag="condps", space="PSUM")
                        for k in range(2):
                            nc.tensor.matmul(ps[:], w2_sb[:, k, oc * 128:(oc + 1) * 128],
                                             c1T[:, k, tb * 512:(tb + 1) * 512],
                                             start=(k == 0), stop=(k == 1))
                        nc.scalar.activation(c2T[:, oc, tb * 512:(tb + 1) * 512], ps[:],
                                             AF.Identity, bias=tb2_sb[:, oc:oc + 1])
                for l in range(6):
                    film_sb = wpool.tile([128, 2, 512], dt.bfloat16, tag="film")
                    nc.sync.dma_start(film_sb[:], film[l * 257:l * 257 + 256, :]
                                      .rearrange("(a p) n -> p a n", p=128))
                    filmb_sb = wpool.tile([1, 512], dt.bfloat16, tag="filmb")
                    nc.sync.dma_start(filmb_sb[:], film[l * 257 + 256:l * 257 + 257, :])
                    for gc in range(8):
                        ps = psC.tile([128, 512], dt.float32, tag="condps", space="PSUM")
                        for k in range(2):
                            nc.tensor.matmul(ps[:], c2T[:, k, gc * 128:(gc + 1) * 128],
                                             film_sb[:, k, :], start=(k == 0), stop=False)
                        nc.tensor.matmul(ps[:], ones1[:], filmb_sb[:],
                                         start=False, stop=True)
                        gbt = tiny.tile([128, 512], dt.bfloat16, tag="gbt")
                        nc.vector.tensor_copy(gbt[:], ps[:])
                        nc.sync.dma_start(
                            gbd[(l * G.B + gc * 128):(l * G.B + (gc + 1) * 128), :], gbt[:])

            # ---- phase B: input stage (aggregates precomputed on host) ----
            for t in range(G.NT):
                ax_sb = tiny.tile([32, 128], dt.bfloat16, tag="axw")
                nc.sync.dma_start(ax_sb[:], axe[0:32, t * 128:(t + 1) * 128])
                ae_sb = tiny.tile([32, 128], dt.bfloat16, tag="aew")
                nc.sync.dma_start(ae_sb[:], axe[32:64, t * 128:(t + 1) * 128])
                zps = psZ.tile([128, 256], dt.float32, tag="z", space="PSUM")
                nc.tensor.matmul(zps[:], ax_sb[:],
                                 win_sb[:32, :], start=True, stop=False)
                nc.tensor.matmul(zps[:], ae_sb[:],
                                 we_sb[:32, :], start=False, stop=False)
                nc.tensor.matmul(zps[:], ones1[:], inb_sb[:], start=False, stop=True)
                h0 = work.tile([128, 256], dt.float32, tag="hn")
                nc.scalar.activation(h0[:], zps[:], AF.Copy)
                nc.sync.dma_start(hA[t * 128:(t + 1) * 128, :], h0[:])
                h0b = work.tile([128, 256], dt.bfloat16, tag="hnb")
                nc.vector.tensor_copy(h0b[:], h0[:])
                nc.sync.dma_start(agin[0][t * 128:(t + 1) * 128, :], h0b[:])

            nc.gpsimd.collective_compute(
                "AllGather", ALU.bypass, replica_groups=RG,
                ins=[agin[0].opt()], outs=[tabs[0].opt()])

            # ---- phase C: 6 layers ----
            with tc.tile_pool(name="psAgg", bufs=2, space="PSUM") as psAgg:
                for l in range(nlayers):
                    table = tabs[l % 2]
                    h_old = hA if l % 2 == 0 else hB
                    h_new = h6_out if l == nlayers - 1 else (hB if l % 2 == 0 else hA)

                    wl_sb = wpool.tile([128, 2, 256], dt.bfloat16, tag="wl")
                    nc.sync.dma_start(wl_sb[:], wconv[l * 256:(l + 1) * 256, :]
                                      .rearrange("(a p) n -> p a n", p=128))
                    bl_sb = wpool.tile([1, 256], dt.bfloat16, tag="bl")
                    nc.sync.dma_start(bl_sb[:], bconv[l:l + 1, :])

                    gb_tiles = {}
                    for cidx in range(13):
                        t0c = cidx * 8
                        ntl = min(8, G.NT - t0c)
                        gbg = gbpool.tile([128, 8, 512], dt.bfloat16, tag="gbg")
                        nc.gpsimd.dma_gather(
                            gbg[:, 0:ntl, :], gbd[l * G.B:(l + 1) * G.B, :],
                            nidx_sb[:, t0c * 8:(t0c + ntl) * 8], ntl * 128, ntl * 128, 512,
                            queue_num=3)
                        for i in range(ntl):
                            gb_tiles[t0c + i] = (gbg, i)

                    qi = 0
                    for (w, ts, calls) in WINS:
                        alo = psAgg.tile([128, 512], dt.float32, tag="alo", space="PSUM")
                        ahi = psAgg.tile([128, 512], dt.float32, tag="ahi", space="PSUM")
                        w_first = calls[0][1]
                        w_last = calls[-1][1] + calls[-1][2] - 1
                        w_ntiles = w_last - w_first + 1
                        s_sb = spool.tile([128, 40 * 128], dt.bfloat16, tag="s")
                        nc.sync.dma_start(s_sb[:, 0:w_ntiles * 128],
                                          S_in[:, w_first * 128:(w_last + 1) * 128])
                        for (b, a, n) in calls:
                            g = gpool.tile([128, G.MAX_TILES_PER_CALL, 256], dt.bfloat16, tag="g")
                            nc.gpsimd.dma_gather(
                                g[:, 0:n, :], table[b * G.BANK:(b + 1) * G.BANK, :],
                                gidx_sb[:, a * 8:(a + n) * 8], n * 128, n * 128, 256,
                                queue_num=qi % 3)
                            qi += 1
                            for i in range(n):
                                ti = a + i
                                seg = int(tile_t[ti]) % G.WLEN
                                st_f = ti == w_first
                                sp_f = ti == w_last
                                sl = ti - w_first
                                nc.tensor.matmul(alo[:, seg * 128:(seg + 1) * 128],
                                                 g[:, i, 0:128],
                                                 s_sb[:, sl * 128:(sl + 1) * 128],
                                                 start=st_f, stop=sp_f)
                                nc.tensor.matmul(ahi[:, seg * 128:(seg + 1) * 128],
                                                 g[:, i, 128:256],
                                                 s_sb[:, sl * 128:(sl + 1) * 128],
                                                 start=st_f, stop=sp_f)
                        for t in ts:
                            seg = t % G.WLEN
                            ab_lo = tiny.tile([128, 128], dt.bfloat16, tag="ablo")
                            nc.vector.tensor_copy(ab_lo[:], alo[:, seg * 128:(seg + 1) * 128])
                            ab_hi = tiny.tile([128, 128], dt.bfloat16, tag="abhi")
                            nc.vector.tensor_copy(ab_hi[:], ahi[:, seg * 128:(seg + 1) * 128])
                            zps = psZ.tile([128, 256], dt.float32, tag="z", space="PSUM")
                            nc.tensor.matmul(zps[:], ab_lo[:], wl_sb[:, 0, :], start=True, stop=False)
                            nc.tensor.matmul(zps[:], ab_hi[:], wl_sb[:, 1, :], start=False, stop=False)
                            nc.tensor.matmul(zps[:], ones1[:], bl_sb[:], start=False, stop=True)
                            s1 = tiny.tile([128, 1], dt.float32, tag="s1")
                            s2 = tiny.tile([128, 1], dt.float32, tag="s2")
                            sq = work.tile([128, 256], dt.float32, tag="sq")
                            nc.scalar.activation(sq[:], zps[:], AF.Square, accum_out=s2[:])
                            zsb = work.tile([128, 256], dt.float32, tag="zsb")
                            nc.scalar.activation(zsb[:], zps[:], AF.Identity, accum_out=s1[:])
                            negmu = tiny.tile([128, 1], dt.float32, tag="negmu")
                            nc.vector.tensor_scalar_mul(negmu[:], s1[:], -1.0 / 256.0)
                            e2 = tiny.tile([128, 1], dt.float32, tag="e2")
                            nc.vector.tensor_scalar_mul(e2[:], s2[:], 1.0 / 256.0)
                            var = tiny.tile([128, 1], dt.float32, tag="var")
                            nc.vector.tensor_tensor(out=var[:], in0=negmu[:], in1=negmu[:], op=ALU.mult)
                            nc.vector.tensor_tensor(out=var[:], in0=e2[:], in1=var[:], op=ALU.subtract)
                            std = tiny.tile([128, 1], dt.float32, tag="std")
                            nc.scalar.activation(std[:], var[:], AF.Sqrt, bias=epsc[:])
                            rstd = tiny.tile([128, 1], dt.float32, tag="rstd")
                            nc.vector.reciprocal(rstd[:], std[:])
                            xh = work.tile([128, 256], dt.float32, tag="xh")
                            nc.vector.tensor_scalar(out=xh[:], in0=zsb[:],
                                                    scalar1=negmu[:], scalar2=rstd[:],
                                                    op0=ALU.add, op1=ALU.mult)
                            gbg, gi = gb_tiles[t]
                            y = work.tile([128, 256], dt.float32, tag="y")
                            nc.vector.tensor_tensor(out=y[:], in0=xh[:], in1=gbg[:, gi, 0:256], op=ALU.mult)
                            nc.vector.tensor_tensor(out=y[:], in0=y[:], in1=gbg[:, gi, 256:512], op=ALU.add)
                            h2 = work.tile([128, 256], dt.float32, tag="h2")
                            nc.scalar.activation(h2[:], y[:], AF.Silu)
                            hold = work.tile([128, 256], dt.float32, tag="hold")
                            nc.sync.dma_start(hold[:], h_old[t * 128:(t + 1) * 128, :])
                            hn = work.tile([128, 256], dt.float32, tag="hn")
                            nc.vector.tensor_tensor(out=hn[:], in0=h2[:], in1=hold[:], op=ALU.add)
                            nc.sync.dma_start(h_new[t * 128:(t + 1) * 128, :], hn[:])
                            if l < nlayers - 1:
                                hnb = work.tile([128, 256], dt.bfloat16, tag="hnb")
                                nc.vector.tensor_copy(hnb[:], hn[:])
                                nc.sync.dma_start(agin[(l + 1) % 2][t * 128:(t + 1) * 128, :], hnb[:])
                    if l < nlayers - 1:
                      with nc.named_scope(f"AG{l}"):
                        nc.gpsimd.collective_compute(
                            "AllGather", ALU.bypass, replica_groups=RG,
                            ins=[agin[(l + 1) % 2].opt()], outs=[tabs[(l + 1) % 2].opt()])

                if nlayers == 0:
                    nc.sync.dma_start(h6_out[:], hA[:])
                if dbg:
                    nc.sync.dma_start(gbd_out[:], gbd[0:G.B, :])

    nc.compile()
    return nc


def make_inputs(inp, st):
    """Build per-core in_maps."""
    n_index = np.asarray(inp["n_index"]).astype(np.int64)
    ntot = st["ntot"]

    xt = np.zeros((G.TROWS, 128), bf16)
    nx = np.asarray(inp["node_x"], np.float32)
    for k in range(NCORES):
        xt[k * G.TR:k * G.TR + G.RPC, :G.NF] = nx[k * G.RPC:(k + 1) * G.RPC].astype(bf16)

    emb = G.sincos_emb(inp["t"])
    embT = np.ascontiguousarray(emb.T).astype(bf16)

    film_tab = G.prep_weights(inp)
    film_dev = film_tab.reshape(6 * 257, 512).astype(bf16)
    wconv = np.asarray(inp["conv2_w"], np.float32).reshape(6 * 256, 256).astype(bf16)
    bconv = np.asarray(inp["conv2_b"], np.float32).astype(bf16)

    w_in32 = np.zeros((32, 256), bf16)
    w_in32[:G.NF] = np.asarray(inp["in_conv_w"], np.float32).astype(bf16)
    we32 = np.zeros((32, 256), bf16)
    we32[:G.EF] = np.asarray(inp["edge_w"], np.float32).astype(bf16)
    we32[G.EF] = np.asarray(inp["edge_b"], np.float32).astype(bf16)
    inb = np.asarray(inp["in_conv_b"], np.float32).astype(bf16)[None, :]

    tb1 = np.ascontiguousarray(np.asarray(inp["t_b1"], np.float32).reshape(2, 128).T)
    tb2 = np.ascontiguousarray(np.asarray(inp["t_b2"], np.float32).reshape(2, 128).T)
    w1 = np.asarray(inp["t_w1"], np.float32).astype(bf16)
    w2 = np.asarray(inp["t_w2"], np.float32).astype(bf16)

    S = G.build_S(st)
    ratio = st["ratio"]
    ee_scaled = np.asarray(inp["edge_e"], np.float32) * ratio[:, None]
    e_aug_full = np.concatenate([ee_scaled, ratio[:, None]], 1).astype(bf16)
    axes = G.host_input_aggregates(inp, st)

    in_maps = []
    for k in range(NCORES):
        S_dev = np.ascontiguousarray(S[k].transpose(1, 0, 2)).reshape(128, ntot * 128)
        ea = np.zeros((ntot, 128, 32), bf16)
        eo = st["eord"][k]
        valid = eo >= 0
        ea[:, :, :17][valid] = e_aug_full[eo[valid]]
        ea_dev = np.ascontiguousarray(ea.transpose(1, 0, 2)).reshape(128, ntot * 32)
        gidx_dev = G.wrap_idx(st["gidx"][k])
        gvals = np.zeros(G.TR, np.int16)
        gvals[:G.RPC] = n_index[k * G.RPC:(k + 1) * G.RPC].astype(np.int16)
        nidx_dev = G.wrap_idx(gvals.reshape(G.NT, 128))
        in_maps.append({
            "xtab": xt, "S_in": S_dev, "eaug": ea_dev, "gidx": gidx_dev,
            "nidx": nidx_dev, "embT": embT, "w1": w1, "w2": w2,
            "tb1": tb1, "tb2": tb2, "film": film_dev, "wconv": wconv,
            "bconv": bconv, "win_w": w_in32, "we_w": we32, "inb": inb,
            "axe": axes[k],
        })
    return in_maps


def run(inp, trace=False, nlayers=6, dbg=False):
    src = np.asarray(inp["src"]).astype(np.int64)
    dst = np.asarray(inp["dst"]).astype(np.int64)
    st = G.build_edge_structure(src, dst)
    nc = build(st, nlayers=nlayers, dbg=dbg)
    in_maps = make_inputs(inp, st)
    res = bass_utils.run_bass_kernel_spmd(
        nc, in_maps, core_ids=list(range(NCORES)), trace=trace,
        trace_cores=[0] if trace else None)
    h6 = np.concatenate([res.results[k]["h6_out"][:G.RPC] for k in range(NCORES)], 0)
    out = G.host_pool_head(h6, np.asarray(inp["n_index"]), inp["head_w"], inp["head_b"])
    return out, res, h6

